# revision 6
# baseline (speedup 1.0000x reference)
"""ColourLoss Trainium2 kernel (self-contained), V3 "PWL relu basis".

Computes, per batch sample b:
    loss[b] = emd(hist_g(img), hist_g(img_t)) + emd(hist_b(img), hist_b(img_t))
(emd(r_hist, r_hist) == 0 exactly, so the r channel is skipped.)

Math: the soft-histogram bin memberships telescope under cumsum:
    cumsum_k pj = sigmoid(2.5*t) - sigmoid(2.5*(t - (k+1))),  t = 255*x
so  cdf[k] = (T0 - F[k+1])/N  with  F[m] = sum_n sigmoid(2.5*(t_n - m)).

F is computed with a radix-16 decomposition: per-pixel block a = floor(t/16),
offset xi = t - 16a in [0,16).  The 22 per-(pixel,slot) sigmoid values of the
previous design are replaced by a piecewise-linear basis: 17 columns
u = [1, relu(xi-0), relu(xi-1), ..., relu(xi-15)] (built on DVE/Act/Pool at
4x tensor_scalar rates instead of 22 scalar-engine sigmoids), binned per
block by a one-hot matmul (tensor engine, contraction over pixels):
    G_basisT[17, 16] += u[128px, 17].T @ onehot[128px, 16]
A constant least-squares matrix W[17, 22] (fit offline against the true
sigmoid slot functions; approximation error largely cancels in the x - y
difference) maps basis sums to the 22 slot sums via one tiny f32 matmul per
pair, applied after differencing the two images.  Saturated (pixel,bin)
pairs are recovered exactly from block counts (suffix sums), identical to
the previous assembly.  Sharding: batch (8 samples) across 8 NeuronCores.
"""
from contextlib import ExitStack

import numpy as np

import concourse.bass as bass
import concourse.tile as tile
from concourse import bacc, bass_utils, mybir

F32 = mybir.dt.float32
F16 = mybir.dt.float16
I32 = mybir.dt.int32

P = 128        # SBUF partitions
FD = 512       # free elems per partition per channel-image (128*512 = 65536)
FDH = 256      # half of FD processed per pipeline unit
NPIX = 65536
NCH = 4        # channel-images per core: [g, g_t, b, b_t]
N_CORES = 8

# Slot columns produced by W: value sigmoid(2.5*(xi - off)).
#   cols 0..15:  off = b          (own block)
#   cols 16..18: off = 16 + b, b in {0,1,2}    (pixel one block below bin)
#   cols 19..21: off = b - 16, b in {13,14,15} (pixel one block above bin)
SLOT_OFFS = [float(b) for b in range(16)] + [16.0, 17.0, 18.0] + [-3.0, -2.0, -1.0]
NSLOT = len(SLOT_OFFS)          # 22
NTAU = 16                       # relu(xi - tau), tau = 0..15
NB = NTAU + 1                   # u columns: [ones, relu(xi-0..15)]

# relu basis columns 2..16 (tau = 1..15) split between engines; col 1 is
# xi itself (= relu(xi - 0)), written directly by the prep chain.
ACT_TAUS = list(range(9, 16))   # 7 columns on the scalar engine
DVE_TAUS = list(range(1, 9))    # 8 columns on DVE
# one-hot columns split: DVE gets 11, Pool (gpsimd) 5
POOL_OH = list(range(11, 16))
DVE_OH = list(range(0, 11))


def build_W() -> np.ndarray:
    """Least-squares fit of the 22 sigmoid slot functions onto the PWL basis
    [1, relu(xi-0), ..., relu(xi-15)] over xi in [0, 16)."""
    xi = np.linspace(0, 16, 4097, endpoint=False) + 16 / 8194.0
    A = np.zeros((xi.size, NB))
    A[:, 0] = 1.0
    for tau in range(NTAU):
        A[:, 1 + tau] = np.maximum(xi - tau, 0.0)
    W = np.zeros((NB, NSLOT), dtype=np.float64)
    for b, off in enumerate(SLOT_OFFS):
        y = 1.0 / (1.0 + np.exp(-2.5 * (xi - off)))
        W[:, b] = np.linalg.lstsq(A, y, rcond=None)[0]
    return W.astype(np.float32)


def _colour_loss_kernel(ctx, tc, out_ap, chs_ap, dbg_ap=None, reps=1):
    nc = tc.nc

    consts = ctx.enter_context(tc.tile_pool(name="consts", bufs=1))
    unitp = ctx.enter_context(tc.tile_pool(name="unitp", bufs=2))
    psums = ctx.enter_context(tc.tile_pool(name="psums", bufs=1, space="PSUM"))
    asm = ctx.enter_context(tc.tile_pool(name="asm", bufs=1))

    # constant W matrix [17, 22] (f32)
    w_dram = nc.inline_tensor(build_W(), name="wmat").ap()
    w_sb = consts.tile([NB, NSLOT], F32)
    nc.sync.dma_start(w_sb[:], w_dram)

    # per-column activation biases (-tau) for the Act-side relu columns
    bias_dram = nc.inline_tensor(
        np.array([[-float(t) for t in ACT_TAUS]], dtype=np.float32), name="actbias"
    ).ap()
    bias_sb = consts.tile([P, len(ACT_TAUS)], F32)
    nc.sync.dma_start(
        bias_sb[:],
        bass.AP(
            tensor=bias_dram.tensor,
            offset=bias_dram.offset,
            ap=[[0, P], bias_dram.ap[1]],
        ),
    )

    pools = (consts, unitp, psums, asm)
    for _ in range(reps):
        _colour_loss_once(tc, out_ap, chs_ap, dbg_ap, pools, w_sb, bias_sb)


def _colour_loss_once(tc, out_ap, chs_ap, dbg_ap, pools, w_sb, bias_sb):
    nc = tc.nc
    AOP = mybir.AluOpType
    ACT = mybir.ActivationFunctionType
    consts, unitp, psums, asm = pools
    FDH2 = 2 * FDH  # two channel-images per unit

    # PSUM accumulators: G_basisT[17, 16] per channel-image
    gps = [
        psums.tile([NB, 16], F32, tag=f"g{i}", name=f"gps{i}") for i in range(NCH)
    ]

    for q in range(2):          # pairs: (g, g_t), (b, b_t)
        for eta in range(2):    # FD halves
            xt = unitp.tile([P, 2, FDH], F32, tag="xt")
            c0 = eta * FDH
            nc.sync.dma_start(xt[:, 0, :], chs_ap[2 * q][:, c0 : c0 + FDH])
            nc.sync.dma_start(xt[:, 1, :], chs_ap[2 * q + 1][:, c0 : c0 + FDH])
            xtf = xt[:].rearrange("p h f -> p (h f)")

            t = unitp.tile([P, FDH2], F32, tag="t")
            nc.vector.tensor_scalar_mul(t[:], xtf, 255.0)
            # a_i = convert(t/16): trunc in sim, round-nearest on HW; the
            # neg-fixup below corrects either to floor.
            a_i = unitp.tile([P, FDH2], I32, tag="a_i")
            nc.vector.tensor_scalar_mul(a_i[:], t[:], 0.0625)
            a_f0 = unitp.tile([P, FDH2], F32, tag="a_f0")
            nc.gpsimd.tensor_copy(a_f0[:], a_i[:])
            xi0 = unitp.tile([P, FDH2], F32, tag="xi0")
            nc.vector.scalar_tensor_tensor(
                xi0[:], in0=a_f0[:], scalar=-16.0, in1=t[:],
                op0=AOP.mult, op1=AOP.add,
            )
            neg = unitp.tile([P, FDH2], F32, tag="neg")
            nc.vector.tensor_scalar(neg[:], xi0[:], 0.0, None, op0=AOP.is_lt)
            a_fx = unitp.tile([P, FDH2], F32, tag="a_fx")
            nc.vector.tensor_tensor(a_fx[:], a_f0[:], neg[:], op=AOP.subtract)
            a_f16 = unitp.tile([P, FDH2], F16, tag="a_f16")
            nc.gpsimd.tensor_copy(a_f16[:], a_fx[:])

            # u basis tile: [ones, relu(xi-0), ..., relu(xi-15)]
            u = unitp.tile([P, NB, FDH2], F16, tag="u")
            nc.gpsimd.memset(u[:, 0, :], 1.0)
            # col 1 = xi = relu(xi - 0), xi >= 0 already
            nc.vector.scalar_tensor_tensor(
                u[:, 1, :], in0=neg[:], scalar=16.0, in1=xi0[:],
                op0=AOP.mult, op1=AOP.add,
            )
            xi16 = u[:, 1, :]
            for tau in DVE_TAUS:
                nc.vector.tensor_scalar(
                    u[:, 1 + tau, :], xi16, float(tau), 0.0,
                    op0=AOP.subtract, op1=AOP.max,
                )
            for i, tau in enumerate(ACT_TAUS):
                nc.scalar.activation(
                    u[:, 1 + tau, :], xi16, ACT.Relu,
                    bias=bias_sb[:, i : i + 1], scale=1.0,
                )

            # one-hot over blocks: ind[p, s, j] = (a[p, j] == s), f16
            ind = unitp.tile([P, 16, FDH2], F16, tag="ind")
            for s in DVE_OH:
                nc.vector.tensor_scalar(
                    ind[:, s, :], a_f16[:], float(s), None, op0=AOP.is_equal
                )
            for s in POOL_OH:
                nc.gpsimd.tensor_scalar(
                    ind[:, s, :], a_f16[:], float(s), None, op0=AOP.is_equal
                )

            # bin: G_basisT[tau, a] += u[px, tau].T @ ind[px, a]
            for h in range(2):
                g = gps[2 * q + h]
                for j in range(FDH):
                    hj = h * FDH + j
                    nc.tensor.matmul(
                        g[:],
                        u[:, :, hj : hj + 1],
                        ind[:, :, hj : hj + 1],
                        start=(eta == 0 and j == 0),
                        stop=(eta == 1 and j == FDH - 1),
                    )

    # ---- per-pair: diff in basis space, apply W, flatten ----
    # dgf layout: [x, pair, slot(22), A(16)] — the final square-sum is
    # order-invariant, so bins are flattened as 16*slot + A throughout.
    dgf = asm.tile([1, 2, NSLOT, 16], F32)
    cnt0 = asm.tile([1, 2, 16], F32)
    for q in range(2):
        gsb0 = asm.tile([NB, 16], F32, tag=f"gsb{2*q}", name=f"gsb{2*q}")
        nc.vector.tensor_copy(gsb0[:], gps[2 * q][:])
        gsb1 = asm.tile([NB, 16], F32, tag=f"gsb{2*q+1}", name=f"gsb{2*q+1}")
        nc.vector.tensor_copy(gsb1[:], gps[2 * q + 1][:])
        gd = asm.tile([NB, 16], F32, tag=f"gd{q}", name=f"gd{q}")
        nc.vector.tensor_tensor(gd[:], gsb0[:], gsb1[:], op=AOP.subtract)
        if dbg_ap is not None:
            nc.sync.dma_start(dbg_ap[q], gd[:])
        sd = psums.tile([NSLOT, 16], F32, tag=f"sd{q}", name=f"sd{q}")
        nc.tensor.matmul(sd[:], w_sb[:], gd[:], start=True, stop=True)
        sd_sb = asm.tile([NSLOT, 16], F32, tag=f"sdsb{q}", name=f"sdsb{q}")
        nc.vector.tensor_copy(sd_sb[:], sd[:])
        nc.sync.dma_start(dgf[:, q], sd_sb[:])
        nc.sync.dma_start(cnt0[:, q], gd[0:1, :])

    # ---- assembly (same math as before, on [slot, A] layout) ----
    cntp = asm.tile([1, 2, 32], F32)
    nc.vector.memset(cntp[:], 0.0)
    nc.vector.tensor_copy(cntp[:, :, 0:16], cnt0[:])
    for k in (1, 2, 4, 8):   # cntp[i] = sum_{j >= i} cnt[j]
        nc.vector.tensor_tensor(
            cntp[:, :, 0 : 32 - k], cntp[:, :, 0 : 32 - k], cntp[:, :, k:32],
            op=AOP.add,
        )

    d = asm.tile([1, 2, 256], F32)
    d4 = d[:].rearrange("x p (b A) -> x p b A", A=16)
    nc.vector.tensor_copy(d4, dgf[:, :, 0:16, :])
    nc.vector.tensor_tensor(
        d4[:, :, 0:3, 1:16], d4[:, :, 0:3, 1:16], dgf[:, :, 16:19, 0:15],
        op=AOP.add,
    )
    nc.vector.tensor_tensor(
        d4[:, :, 13:16, 0:15], d4[:, :, 13:16, 0:15], dgf[:, :, 19:22, 1:16],
        op=AOP.add,
    )
    nc.vector.tensor_tensor(
        d4, d4, cntp[:, :, 2:18].unsqueeze(2).broadcast_to([1, 2, 16, 16]),
        op=AOP.add,
    )
    nc.vector.tensor_tensor(
        d4[:, :, 0:13, 0:15],
        d4[:, :, 0:13, 0:15],
        cnt0[:, :, 1:16].unsqueeze(2).broadcast_to([1, 2, 13, 15]),
        op=AOP.add,
    )

    # T0 diff per pair: T0 = (N - cnt[0]) + G[0, col0]
    t0d = asm.tile([1, 2, 1], F32)
    nc.vector.tensor_tensor(
        t0d[:], dgf[:, :, 0, 0:1], cnt0[:, :, 0:1], op=AOP.subtract
    )
    invn = 1.0 / float(NPIX)
    t0dn = asm.tile([1, 2, 1], F32)
    nc.vector.tensor_scalar_mul(t0dn[:], t0d[:], invn)

    tmp = asm.tile([1, 2, 256], F32)
    nc.vector.scalar_tensor_tensor(
        tmp[:], in0=d[:], scalar=-invn, in1=t0dn[:].broadcast_to([1, 2, 256]),
        op0=AOP.mult, op1=AOP.add,
    )
    dummy = asm.tile([1, 2, 255], F32)
    lossacc = asm.tile([1, 1], F32)
    nc.scalar.activation(
        dummy[:], tmp[:, :, 1:256], ACT.Square, accum_out=lossacc[:]
    )
    nc.sync.dma_start(out_ap[:], lossacc[:])


_CACHE: dict = {}


def build_nc(reps: int = 1, dbg: bool = False):
    key = ("nc", reps, dbg)
    if key in _CACHE:
        return _CACHE[key]
    nc = bacc.Bacc(
        "TRN2", target_bir_lowering=False, debug=False, num_devices=N_CORES
    )
    chs = nc.dram_tensor("chs", [NCH, P, FD], F32, kind="ExternalInput").ap()
    out = nc.dram_tensor("out", [1, 1], F32, kind="ExternalOutput").ap()
    dbg_ap = None
    if dbg:
        dbg_ap = nc.dram_tensor("dbg", [2, NB, 16], F32, kind="ExternalOutput").ap()
    with tile.TileContext(nc) as tc:
        with ExitStack() as ctx:
            _colour_loss_kernel(ctx, tc, out, chs, dbg_ap, reps=reps)
    nc.compile()
    _CACHE[key] = nc
    return nc


def make_in_maps(img: np.ndarray, img_t: np.ndarray):
    img = np.asarray(img)
    img_t = np.asarray(img_t)
    in_maps = []
    for c in range(N_CORES):
        chs = np.stack(
            [
                img[c, 1].reshape(P, FD),
                img_t[c, 1].reshape(P, FD),
                img[c, 2].reshape(P, FD),
                img_t[c, 2].reshape(P, FD),
            ],
            axis=0,
        ).astype(np.float32)
        in_maps.append({"chs": np.ascontiguousarray(chs)})
    return in_maps


def kernel(img: np.ndarray, img_t: np.ndarray, trace: bool = False):
    nc = build_nc()
    in_maps = make_in_maps(img, img_t)
    res = bass_utils.run_bass_kernel_spmd(
        nc, in_maps, core_ids=list(range(N_CORES)), trace=trace
    )
    out = np.array(
        [res.results[c]["out"][0, 0] for c in range(N_CORES)], dtype=np.float32
    )
    if trace:
        kernel.last_results = res  # type: ignore[attr-defined]
    return out
